# revision 6
# baseline (speedup 1.0000x reference)
"""Trainium2 Bass kernel v2 for nn_Decoder_arch2 (LSTM image-caption decoder).

Reference computation (B=128, T=24 used steps, E=512, H2=1024, V=30000):
  tok = emb[captions]; seq = [pad_emb, tok[:, :23]]           # [B, 24, E]
  x_t = concat(seq_t, features)                               # [B, 2E]
  xg = x @ W_ih.T + b_ih + b_hh                               # [B, 24, 4096]
  24x LSTMCell steps (c = f*c + i*tanh(g); h = o*tanh(c))
  logits_t = h_t @ W_out.T + b_out                            # [B, 24, V]
  out = transpose(logits, (0, 2, 1))                          # [B, V, 24]

Sharding: batch-parallel LSTM (16 rows/core) + vocab-parallel projection
(3840 vocab rows/core) bridged by a chunked AllGather of the hidden states.
All pools coexist in one scope so projection matmuls fill PE gaps during the
LSTM as soon as each AllGather chunk lands.

Per-core phases:
  1. xg GEMM in transposed orientation: stationary = x.T blocks packed as
     [e-chunk, 8t x 16b], moving = W_ih.T (streamed in 512-col chunks) ->
     xgT [128(t,b), 3 tblk, 4096], bias folded in via a K=1 ones-row matmul.
     W_ih/bias pre-scaled by 2^14 so the fp8 recurrence shares one descale.
  2. LSTM in transposed orientation: per step, gates.T [16b, 4096g] accumulate
     in PSUM octants [16, 512]: one identity-matmul injects the xgT slice,
     then 4 fp8 DoubleRow matmuls (h chunk-pairs stationary, W_hh.T moving)
     add the recurrent term. W_hh is pre-scaled x256 and h x64 to keep fp8
     e4m3 out of its subnormal range; the gate activations descale by 2^-14.
     h_t is transposed back to [128(hu), 16b] via 8 PE transposes feeding the
     next step's stationaries (fp8) and the hs buffer (bf16).
  3. AllGather of hs in 6 t-chunks (every 4 steps) across the 8 cores.
  4. Projection in 6 t-phases: phase ph needs only AllGather chunk ph, so
     most phases overlap the LSTM. W_out tiles streamed per (phase, vt).

Host does all layout prep (gather, transposes, casts, scaling) and output
assembly.
"""

import sys

if "/opt/trn_rl_repo" not in sys.path:
    sys.path.insert(0, "/opt/trn_rl_repo")

import numpy as np
import ml_dtypes

import concourse.bass as bass
import concourse.bacc as bacc
import concourse.mybir as mybir
import concourse.tile as tile
from concourse.bass_utils import run_bass_kernel_spmd

bf16 = ml_dtypes.bfloat16
fp8 = ml_dtypes.float8_e4m3
F32 = mybir.dt.float32
BF16 = mybir.dt.bfloat16
FP8 = mybir.dt.float8e4

B, T, E, V, H2 = 128, 24, 512, 30000, 1024
G = 4 * H2  # 4096
NC_N = 8
BS = B // NC_N  # 16 batch rows per core
TB = 3  # t-blocks of 8 timesteps (8t x 16b = 128 partitions)
KC = 6  # AllGather / projection chunks of TPC timesteps
TPC = T // KC  # 4
NVT = 240  # padded vocab tiles total (30720 rows)
VP = NVT * 128
NVT_C = NVT // NC_N  # 30 vt tiles per core
VSH = NVT_C * 128  # 3840 vocab rows per core

WSC = 256.0  # W_hh fp8 pre-scale
HSC = 64.0  # h fp8 pre-scale
SC = WSC * HSC  # total gate pre-activation scale (W_ih/bias pre-scaled by SC)

SIG = mybir.ActivationFunctionType.Sigmoid
TANH = mybir.ActivationFunctionType.Tanh
COPY = mybir.ActivationFunctionType.Copy
IDENT = mybir.ActivationFunctionType.Identity
DR = mybir.MatmulPerfMode.DoubleRow


def _emit_quarter(nc, po_, id16_sb, xg_t, h8_prev, whh_sb, q, t):
    """Emit one gate quarter: per 512-col half (one PSUM bank), an identity
    matmul injecting the xg slice plus 4 fp8 DoubleRow recurrent matmuls."""
    for half in range(2):
        cs = slice(q * 1024 + half * 512, q * 1024 + (half + 1) * 512)
        hs_ = slice(half * 512, (half + 1) * 512)
        nc.tensor.matmul(
            po_[:, hs_], id16_sb[:], xg_t[:, cs], start=True, stop=(t == 0)
        )
        if t == 0:
            continue
        for j in range(4):
            nc.tensor.matmul(
                po_[:, hs_],
                h8_prev[:, 2 * j : 2 * j + 2, :],
                whh_sb[:, j, :, cs],
                start=False,
                stop=(j == 3),
                perf_mode=DR,
            )


def build_nc():
    nc = bacc.Bacc(None, target_bir_lowering=False)

    xstat_d = nc.dram_tensor("xstat", [128, TB, 4, 128], BF16, kind="ExternalInput")
    featst_d = nc.dram_tensor("featst", [128, 4, 128], BF16, kind="ExternalInput")
    wihT_d = nc.dram_tensor("wihT", [128, 8, G], BF16, kind="ExternalInput")
    whh8_d = nc.dram_tensor("whh8", [128, 4, 2, G], FP8, kind="ExternalInput")
    brow_d = nc.dram_tensor("brow", [1, G], BF16, kind="ExternalInput")
    ones_d = nc.dram_tensor("onesrow", [1, 128], BF16, kind="ExternalInput")
    id16_d = nc.dram_tensor("ident16", [16, 16], BF16, kind="ExternalInput")
    wop_d = nc.dram_tensor("wop", [NVT_C, 128, 8, 128], BF16, kind="ExternalInput")
    boutT_d = nc.dram_tensor("boutT", [128, NVT_C], F32, kind="ExternalInput")
    # one output tensor per t-phase of TPC steps; cols = (r, t_in_phase, b)
    out_d = [
        nc.dram_tensor(f"out{ph}", [NVT_C, 128, NC_N, TPC, BS], BF16,
                       kind="ExternalOutput")
        for ph in range(KC)
    ]

    with tile.TileContext(nc) as tc:
        with (
            tc.tile_pool(name="const", bufs=1) as const,
            tc.tile_pool(name="hs", bufs=1) as hsp,
            tc.tile_pool(name="hsall", bufs=3) as hap,
            tc.tile_pool(name="xg", bufs=1) as xgp,
            tc.tile_pool(name="wih", bufs=1) as wihp,
            tc.tile_pool(name="brc", bufs=1) as brcp,
            tc.tile_pool(name="whh", bufs=1) as whhp,
            tc.tile_pool(name="cst", bufs=1) as cstp,
            tc.tile_pool(name="tmp", bufs=1) as tmp,
            tc.tile_pool(name="h8", bufs=2) as h8p,
            tc.tile_pool(name="xgt", bufs=1) as xgtp,
            tc.tile_pool(name="wo", bufs=1) as wo_p,
            tc.tile_pool(name="stg", bufs=3) as stgp,
            tc.tile_pool(name="dram", bufs=1, space="DRAM") as dram,
            tc.tile_pool(name="ps", bufs=2, space="PSUM") as psp,
            tc.tile_pool(name="po", bufs=3, space="PSUM") as pop,
            tc.tile_pool(name="ptr", bufs=1, space="PSUM") as ptrp,
        ):
            xstat_sb = const.tile([128, TB, 4, 128], BF16)
            nc.sync.dma_start(xstat_sb[:], xstat_d[:])
            featst_sb = const.tile([128, 4, 128], BF16)
            nc.sync.dma_start(featst_sb[:], featst_d[:])
            ones_sb = const.tile([1, 128], BF16)
            nc.sync.dma_start(ones_sb[:], ones_d[:])
            id16_sb = const.tile([16, 16], BF16)
            nc.sync.dma_start(id16_sb[:], id16_d[:])
            boutT_sb = const.tile([128, NVT_C], F32)
            nc.sync.dma_start(boutT_sb[:], boutT_d[:])

            hs_sb = hsp.tile([128, 8, T, BS], BF16)  # [hu, hc, t, b]
            xgT = xgp.tile([128, TB, G], BF16)  # [8t*16b, tblk, g] (x SC)
            whh_sb = whhp.tile([128, 4, 2, G], FP8)
            nc.sync.dma_start(whh_sb[:], whh8_d[:])
            c_sb = cstp.tile([16, H2], F32)
            # all W_out tiles as one resident tile, loaded once on the
            # scalar ring so the sync ring stays free for xg staging
            wop_all = wo_p.tile([128, NVT_C, 8, 128], BF16)
            nc.scalar.dma_start(
                wop_all[:], wop_d[:].rearrange("vt p hc j -> p vt hc j")
            )
            hs_all_t = []  # per-AG-chunk gathered hs tiles (ring of 4)
            ag_in = [
                dram.tile([128, 8, TPC, BS], BF16, name=f"agi{k}") for k in range(KC)
            ]
            ag_out = [
                dram.tile([NC_N * 128, 8, TPC, BS], BF16, addr_space="Shared",
                          name=f"ago{k}")
                for k in range(KC)
            ]

            # ---- phase 1: xg GEMM (transposed orientation), W_ih streamed ----
            for cc in range(8):
                ccs = slice(cc * 512, (cc + 1) * 512)
                wih_c = wihp.tile([128, 8, 512], BF16, tag="wih")
                nc.sync.dma_start(wih_c[:], wihT_d[:, :, ccs])
                brow_c = brcp.tile([1, 512], BF16, tag="brc")
                nc.sync.dma_start(brow_c[:], brow_d[:, ccs])
                for tblk in range(TB):
                    px = psp.tile([128, 512], F32, tag="ps")
                    for ec in range(4):
                        nc.tensor.matmul(
                            px[:],
                            xstat_sb[:, tblk, ec],
                            wih_c[:, ec, :],
                            start=(ec == 0),
                            stop=False,
                        )
                    for ec in range(4):
                        nc.tensor.matmul(
                            px[:],
                            featst_sb[:, ec],
                            wih_c[:, 4 + ec, :],
                            start=False,
                            stop=False,
                        )
                    nc.tensor.matmul(
                        px[:], ones_sb[:], brow_c[:], start=False, stop=True
                    )
                    nc.scalar.activation(xgT[:, tblk, ccs], px[:], COPY)

            # ---- phase 2: LSTM (transposed orientation, fp8 recurrence) ----
            # gate quarters: q0=i, q1=f, q2=g, q3=o. Emit f, i, g, o:
            # o is only needed by the final h multiply, so it goes last;
            # the c-chain (needs f, i, g) starts as early as possible.
            Q_ORDER = [1, 0, 2, 3]
            for t in range(T):
                tblk, p0 = t // 8, (t % 8) * BS
                # stage this step's xg slice down to partition base 0
                # (PE operands require base partition 0/32/64)
                xg_t = xgtp.tile([16, G], BF16, tag="xgt")
                nc.sync.dma_start(xg_t[:], xgT[p0 : p0 + BS, tblk, :])

                t_i = tmp.tile([16, H2], F32, tag="ti")
                t_f = tmp.tile([16, H2], F32, tag="tf")
                t_g = tmp.tile([16, H2], F32, tag="tg")
                t_o = tmp.tile([16, H2], F32, tag="to")
                gate_tmp = {0: t_i, 1: t_f, 2: t_g, 3: t_o}

                for q in Q_ORDER:
                    if t == 0 and q == 1:
                        continue  # f unused at t=0 (c_0 = 0)
                    po_ = psp.tile([16, 1024], F32, tag="ps")
                    h8p_ = h8_prev if t > 0 else None
                    _emit_quarter(nc, po_, id16_sb, xg_t, h8p_, whh_sb, q, t)
                    func = TANH if q == 2 else SIG
                    nc.scalar.activation(
                        gate_tmp[q][:], po_[:], func, scale=1.0 / SC,
                    )

                t_c = tmp.tile([16, H2], F32, tag="tc2")
                hT = tmp.tile([16, H2], BF16, tag="hT2")
                for hf in range(2):
                    s = slice(hf * 512, (hf + 1) * 512)
                    if t == 0:
                        nc.vector.tensor_mul(c_sb[:, s], t_i[:, s], t_g[:, s])
                    else:
                        nc.vector.tensor_mul(t_f[:, s], t_f[:, s], c_sb[:, s])
                        nc.vector.tensor_mul(t_i[:, s], t_i[:, s], t_g[:, s])
                        nc.vector.tensor_add(c_sb[:, s], t_f[:, s], t_i[:, s])
                    nc.scalar.activation(t_c[:, s], c_sb[:, s], TANH)
                    nc.vector.tensor_mul(hT[:, s], t_o[:, s], t_c[:, s])

                ptr = ptrp.tile([128, 8, BS], BF16, tag="ptr")
                for hc in range(8):
                    nc.tensor.transpose(
                        ptr[:, hc], hT[:, hc * 128 : (hc + 1) * 128], id16_sb[:]
                    )
                nc.vector.tensor_copy(hs_sb[:, :, t, :], ptr[:])
                h8_prev = h8p.tile([128, 8, BS], FP8, tag="h8")
                nc.vector.tensor_scalar_mul(h8_prev[:], ptr[:], HSC)

                # ---- phase 3 (interleaved): chunked AllGather of hs ----
                if t % TPC == TPC - 1:
                    k = t // TPC
                    ts = slice(k * TPC, (k + 1) * TPC)
                    nc.gpsimd.dma_start(out=ag_in[k][:], in_=hs_sb[:, :, ts, :])
                    nc.gpsimd.collective_compute(
                        "AllGather",
                        mybir.AluOpType.bypass,
                        replica_groups=[list(range(NC_N))],
                        ins=[ag_in[k].opt()],
                        outs=[ag_out[k].opt()],
                    )
                    hs_k = hap.tile([128, 8, NC_N, TPC, BS], BF16, tag="ha")
                    hs_all_t.append(hs_k)
                    nc.gpsimd.dma_start(
                        out=hs_k[:],
                        in_=ag_out[k][:].rearrange(
                            "(r p) hc t b -> p hc r t b", p=128
                        ),
                    )

            # ---- phase 4: vocab-sharded projection, pipelined per t-phase ----
            # t-phase ph only needs AllGather chunk ph, so early phases overlap
            # the LSTM; W_out tiles are re-streamed per (phase, vt).
            for ph in range(KC):
                for vt in range(NVT_C):
                    po = pop.tile([128, NC_N, TPC, BS], F32, tag="po")
                    for hc in range(8):
                        nc.tensor.matmul(
                            po[:],
                            wop_all[:, vt, hc],
                            hs_all_t[ph][:, hc],
                            start=(hc == 0),
                            stop=(hc == 7),
                        )
                    st = stgp.tile([128, NC_N, TPC, BS], BF16, tag="st")
                    nc.vector.tensor_scalar_add(
                        st[:], po[:], boutT_sb[:, vt : vt + 1]
                    )
                    eng = nc.scalar if ph < 3 else nc.sync
                    eng.dma_start(out_d[ph][vt], st[:])

    nc.compile()
    return nc


def prep_host(features, captions, pad_idx, emb, W_ih, W_hh, b_ih, b_hh, W_out, b_out):
    """Host-side layout prep. Returns (shared dict, per-core list of dicts)."""
    features = np.asarray(features, dtype=np.float32)
    captions = np.asarray(captions).astype(np.int64)
    pad_idx = int(np.asarray(pad_idx))
    emb = np.asarray(emb, dtype=np.float32)
    W_ih = np.asarray(W_ih, dtype=np.float32)
    W_hh = np.asarray(W_hh, dtype=np.float32)
    bsum = np.asarray(b_ih, dtype=np.float32) + np.asarray(b_hh, dtype=np.float32)
    W_out = np.asarray(W_out, dtype=np.float32)
    b_out = np.asarray(b_out, dtype=np.float32)

    # seqtok[t, b]: pad for t=0, captions[b, t-1] for t>=1
    seqtok = np.empty((T, B), np.int64)
    seqtok[0, :] = pad_idx
    seqtok[1:, :] = captions[:, : T - 1].T
    xtok = emb[seqtok]  # [T, B, E]

    # wihT[p, ec, g] = W_ih[g, ec*128+p] * SC
    wihT = np.ascontiguousarray(
        (W_ih * SC).T.reshape(8, 128, G).transpose(1, 0, 2).astype(bf16)
    )
    # whh8[p, j, kt, g] = W_hh[g, (2j+kt)*128+p] * WSC
    whh8 = np.ascontiguousarray(
        (W_hh * WSC).T.reshape(4, 2, 128, G).transpose(2, 0, 1, 3).astype(fp8)
    )
    brow = np.ascontiguousarray((bsum * SC)[None, :].astype(bf16))
    onesrow = np.ones((1, 128), bf16)
    ident16 = np.eye(16, dtype=bf16)

    Wout_pad = np.zeros((VP, H2), np.float32)
    Wout_pad[:V] = W_out
    bout_pad = np.zeros((VP,), np.float32)
    bout_pad[:V] = b_out

    shared = {"wihT": wihT, "whh8": whh8, "brow": brow, "onesrow": onesrow,
              "ident16": ident16}

    per_core = []
    for c in range(NC_N):
        bsl = slice(c * BS, (c + 1) * BS)
        # xstat[p, tblk, ec, ti*16+bl] = xtok[tblk*8+ti, c*16+bl, ec*128+p]
        xs = xtok[:, bsl, :]  # [24, 16, 512]
        xs = xs.reshape(TB, 8, BS, 4, 128)  # [tblk, ti, bl, ec, p]
        xstat = np.ascontiguousarray(
            xs.transpose(4, 0, 3, 1, 2).reshape(128, TB, 4, 8 * BS).astype(bf16)
        )
        # featst[p, ec, ti*16+bl] = features[c*16+bl, ec*128+p]
        f = features[bsl].reshape(BS, 4, 128)  # [bl, ec, p]
        featst = np.ascontiguousarray(
            np.broadcast_to(
                f.transpose(2, 1, 0)[:, :, None, :], (128, 4, 8, BS)
            ).reshape(128, 4, 128).astype(bf16)
        )
        # wop[vt, p(hu), hc, j] = Wout_pad[c*3840 + vt*128 + j, hc*128 + p]
        w = Wout_pad[c * VSH : (c + 1) * VSH].reshape(NVT_C, 128, 8, 128)
        wop = np.ascontiguousarray(w.transpose(0, 3, 2, 1).astype(bf16))
        # boutT[p, vt] = bout_pad[c*3840 + vt*128 + p]
        bT = np.ascontiguousarray(
            bout_pad[c * VSH : (c + 1) * VSH].reshape(NVT_C, 128).T
        )
        per_core.append({"xstat": xstat, "featst": featst, "wop": wop, "boutT": bT})
    return shared, per_core


_NC_CACHE = None


def kernel(**inputs) -> np.ndarray:
    global _NC_CACHE
    if _NC_CACHE is None:
        _NC_CACHE = build_nc()
    nc = _NC_CACHE

    shared, per_core = prep_host(**inputs)
    in_maps = [dict(shared, **pc) for pc in per_core]
    res = run_bass_kernel_spmd(nc, in_maps, core_ids=list(range(NC_N)))

    out = np.empty((B, VP, T), np.float32)
    for c in range(NC_N):
        for ph in range(KC):
            o = np.asarray(res.results[c][f"out{ph}"])  # [30, 128, 8r, TPCt, 16b]
            a = o.astype(np.float32).transpose(2, 4, 0, 1, 3)  # [r, bl, vt, j, tp]
            out[:, c * VSH : (c + 1) * VSH, ph * TPC : (ph + 1) * TPC] = a.reshape(
                B, VSH, TPC
            )
    return out[:, :V, :]


# revision 7
# speedup vs baseline: 1.4934x; 1.4934x over previous
"""Trainium2 Bass kernel v2 for nn_Decoder_arch2 (LSTM image-caption decoder).

Reference computation (B=128, T=24 used steps, E=512, H2=1024, V=30000):
  tok = emb[captions]; seq = [pad_emb, tok[:, :23]]           # [B, 24, E]
  x_t = concat(seq_t, features)                               # [B, 2E]
  xg = x @ W_ih.T + b_ih + b_hh                               # [B, 24, 4096]
  24x LSTMCell steps (c = f*c + i*tanh(g); h = o*tanh(c))
  logits_t = h_t @ W_out.T + b_out                            # [B, 24, V]
  out = transpose(logits, (0, 2, 1))                          # [B, V, 24]

Sharding: batch-parallel LSTM (16 rows/core) + vocab-parallel projection
(3840 vocab rows/core) bridged by a chunked AllGather of the hidden states.
All pools coexist in one scope so projection matmuls fill PE gaps during the
LSTM as soon as each AllGather chunk lands.

Per-core phases:
  1. xg GEMM in transposed orientation: stationary = x.T blocks packed as
     [e-chunk, 8t x 16b], moving = W_ih.T (streamed in 512-col chunks) ->
     xgT [128(t,b), 3 tblk, 4096], bias folded in via a K=1 ones-row matmul.
     W_ih/bias pre-scaled by 2^14 so the fp8 recurrence shares one descale.
  2. LSTM in transposed orientation: per step, gates.T [16b, 4096g] accumulate
     in PSUM octants [16, 512]: one identity-matmul injects the xgT slice,
     then 4 fp8 DoubleRow matmuls (h chunk-pairs stationary, W_hh.T moving)
     add the recurrent term. W_hh is pre-scaled x256 and h x64 to keep fp8
     e4m3 out of its subnormal range; the gate activations descale by 2^-14.
     h_t is transposed back to [128(hu), 16b] via 8 PE transposes feeding the
     next step's stationaries (fp8) and the hs buffer (bf16).
  3. AllGather of hs in 6 t-chunks (every 4 steps) across the 8 cores.
  4. Projection in 6 t-phases: phase ph needs only AllGather chunk ph, so
     most phases overlap the LSTM. W_out tiles streamed per (phase, vt).

Host does all layout prep (gather, transposes, casts, scaling) and output
assembly.
"""

import sys

if "/opt/trn_rl_repo" not in sys.path:
    sys.path.insert(0, "/opt/trn_rl_repo")

import numpy as np
import ml_dtypes

import concourse.bass as bass
import concourse.bacc as bacc
import concourse.mybir as mybir
import concourse.tile as tile
from concourse.bass_utils import run_bass_kernel_spmd

bf16 = ml_dtypes.bfloat16
fp8 = ml_dtypes.float8_e4m3
F32 = mybir.dt.float32
BF16 = mybir.dt.bfloat16
FP8 = mybir.dt.float8e4

B, T, E, V, H2 = 128, 24, 512, 30000, 1024
G = 4 * H2  # 4096
NC_N = 8
BS = B // NC_N  # 16 batch rows per core
TB = 3  # t-blocks of 8 timesteps (8t x 16b = 128 partitions)
KC = 6  # AllGather / projection chunks of TPC timesteps
TPC = T // KC  # 4
NVT = 240  # padded vocab tiles total (30720 rows)
VP = NVT * 128
NVT_C = NVT // NC_N  # 30 vt tiles per core
VSH = NVT_C * 128  # 3840 vocab rows per core

WSC = 256.0  # W_hh fp8 pre-scale
HSC = 64.0  # h fp8 pre-scale
SC = WSC * HSC  # total gate pre-activation scale (W_ih/bias pre-scaled by SC)

SIG = mybir.ActivationFunctionType.Sigmoid
TANH = mybir.ActivationFunctionType.Tanh
COPY = mybir.ActivationFunctionType.Copy
IDENT = mybir.ActivationFunctionType.Identity
DR = mybir.MatmulPerfMode.DoubleRow


def _emit_quarter(nc, po_, id16_sb, xg_t, h8_prev, whh_sb, q, t):
    """Emit one gate quarter: per 512-col half (one PSUM bank), an identity
    matmul injecting the xg slice plus 4 fp8 DoubleRow recurrent matmuls."""
    for half in range(2):
        cs = slice(q * 1024 + half * 512, q * 1024 + (half + 1) * 512)
        hs_ = slice(half * 512, (half + 1) * 512)
        nc.tensor.matmul(
            po_[:, hs_], id16_sb[:], xg_t[:, cs], start=True, stop=(t == 0)
        )
        if t == 0:
            continue
        for j in range(4):
            nc.tensor.matmul(
                po_[:, hs_],
                h8_prev[:, 2 * j : 2 * j + 2, :],
                whh_sb[:, j, :, cs],
                start=False,
                stop=(j == 3),
                perf_mode=DR,
            )


def build_nc():
    nc = bacc.Bacc(None, target_bir_lowering=False)

    xstat_d = nc.dram_tensor("xstat", [128, TB, 4, 128], BF16, kind="ExternalInput")
    featst_d = nc.dram_tensor("featst", [128, 4, 128], BF16, kind="ExternalInput")
    wihT_d = nc.dram_tensor("wihT", [128, 8, G], BF16, kind="ExternalInput")
    whh8_d = nc.dram_tensor("whh8", [128, 4, 2, G], FP8, kind="ExternalInput")
    brow_d = nc.dram_tensor("brow", [1, G], BF16, kind="ExternalInput")
    ones_d = nc.dram_tensor("onesrow", [1, 128], BF16, kind="ExternalInput")
    id16_d = nc.dram_tensor("ident16", [16, 16], BF16, kind="ExternalInput")
    wop_d = nc.dram_tensor("wop", [NVT_C, 128, 8, 128], BF16, kind="ExternalInput")
    boutT_d = nc.dram_tensor("boutT", [128, NVT_C], F32, kind="ExternalInput")
    # one output tensor per t-phase of TPC steps; cols = (r, t_in_phase, b)
    out_d = [
        nc.dram_tensor(f"out{ph}", [NVT_C, 128, NC_N, TPC, BS], BF16,
                       kind="ExternalOutput")
        for ph in range(KC)
    ]

    with tile.TileContext(nc) as tc:
        with (
            tc.tile_pool(name="const", bufs=1) as const,
            tc.tile_pool(name="hs", bufs=1) as hsp,
            tc.tile_pool(name="hsall", bufs=3) as hap,
            tc.tile_pool(name="xg", bufs=1) as xgp,
            tc.tile_pool(name="wih", bufs=1) as wihp,
            tc.tile_pool(name="brc", bufs=1) as brcp,
            tc.tile_pool(name="whh", bufs=1) as whhp,
            tc.tile_pool(name="cst", bufs=1) as cstp,
            tc.tile_pool(name="tmp", bufs=1) as tmp,
            tc.tile_pool(name="h8", bufs=2) as h8p,
            tc.tile_pool(name="xgt", bufs=1) as xgtp,
            tc.tile_pool(name="wo", bufs=1) as wo_p,
            tc.tile_pool(name="stg", bufs=3) as stgp,
            tc.tile_pool(name="dram", bufs=1, space="DRAM") as dram,
            tc.tile_pool(name="ps", bufs=2, space="PSUM") as psp,
            tc.tile_pool(name="po", bufs=3, space="PSUM") as pop,
            tc.tile_pool(name="ptr", bufs=1, space="PSUM") as ptrp,
        ):
            xstat_sb = const.tile([128, TB, 4, 128], BF16)
            nc.sync.dma_start(xstat_sb[:], xstat_d[:])
            featst_sb = const.tile([128, 4, 128], BF16)
            nc.sync.dma_start(featst_sb[:], featst_d[:])
            ones_sb = const.tile([1, 128], BF16)
            nc.sync.dma_start(ones_sb[:], ones_d[:])
            id16_sb = const.tile([16, 16], BF16)
            nc.sync.dma_start(id16_sb[:], id16_d[:])
            boutT_sb = const.tile([128, NVT_C], F32)
            nc.sync.dma_start(boutT_sb[:], boutT_d[:])

            hs_sb = hsp.tile([128, 8, T, BS], BF16)  # [hu, hc, t, b]
            xgT = xgp.tile([128, TB, G], BF16)  # [8t*16b, tblk, g] (x SC)
            whh_sb = whhp.tile([128, 4, 2, G], FP8)
            nc.sync.dma_start(whh_sb[:], whh8_d[:])
            c_sb = cstp.tile([16, H2], F32)
            # all W_out tiles as one resident tile, loaded once on the
            # scalar ring so the sync ring stays free for xg staging
            wop_all = wo_p.tile([128, NVT_C, 8, 128], BF16)
            nc.scalar.dma_start(
                wop_all[:], wop_d[:].rearrange("vt p hc j -> p vt hc j")
            )
            hs_all_t = []  # per-AG-chunk gathered hs tiles (ring of 4)
            ag_in = [
                dram.tile([128, 8, TPC, BS], BF16, name=f"agi{k}") for k in range(KC)
            ]
            ag_out = [
                dram.tile([NC_N * 128, 8, TPC, BS], BF16, addr_space="Shared",
                          name=f"ago{k}")
                for k in range(KC)
            ]

            # ---- phase 1: xg GEMM (transposed orientation), W_ih streamed ----
            for cc in range(8):
                ccs = slice(cc * 512, (cc + 1) * 512)
                wih_c = wihp.tile([128, 8, 512], BF16, tag="wih")
                nc.sync.dma_start(wih_c[:], wihT_d[:, :, ccs])
                brow_c = brcp.tile([1, 512], BF16, tag="brc")
                nc.sync.dma_start(brow_c[:], brow_d[:, ccs])
                for tblk in range(TB):
                    px = psp.tile([128, 512], F32, tag="ps")
                    for ec in range(4):
                        nc.tensor.matmul(
                            px[:],
                            xstat_sb[:, tblk, ec],
                            wih_c[:, ec, :],
                            start=(ec == 0),
                            stop=False,
                        )
                    for ec in range(4):
                        nc.tensor.matmul(
                            px[:],
                            featst_sb[:, ec],
                            wih_c[:, 4 + ec, :],
                            start=False,
                            stop=False,
                        )
                    nc.tensor.matmul(
                        px[:], ones_sb[:], brow_c[:], start=False, stop=True
                    )
                    nc.scalar.activation(xgT[:, tblk, ccs], px[:], COPY)

            # ---- phase 2: LSTM (transposed orientation, fp8 recurrence) ----
            # gate quarters: q0=i, q1=f, q2=g, q3=o. Emit f, i, g, o:
            # o is only needed by the final h multiply, so it goes last;
            # the c-chain (needs f, i, g) starts as early as possible.
            Q_ORDER = [1, 0, 2, 3]
            for t in range(T):
                tblk, p0 = t // 8, (t % 8) * BS
                # stage this step's xg slice down to partition base 0
                # (PE operands require base partition 0/32/64)
                xg_t = xgtp.tile([16, G], BF16, tag="xgt")
                nc.sync.dma_start(xg_t[:], xgT[p0 : p0 + BS, tblk, :])

                t_i = tmp.tile([16, H2], F32, tag="ti")
                t_f = tmp.tile([16, H2], F32, tag="tf")
                t_g = tmp.tile([16, H2], F32, tag="tg")
                t_o = tmp.tile([16, H2], F32, tag="to")
                gate_tmp = {0: t_i, 1: t_f, 2: t_g, 3: t_o}

                for q in Q_ORDER:
                    if t == 0 and q == 1:
                        continue  # f unused at t=0 (c_0 = 0)
                    po_ = psp.tile([16, 1024], F32, tag="ps")
                    h8p_ = h8_prev if t > 0 else None
                    _emit_quarter(nc, po_, id16_sb, xg_t, h8p_, whh_sb, q, t)
                    func = TANH if q == 2 else SIG
                    nc.scalar.activation(
                        gate_tmp[q][:], po_[:], func, scale=1.0 / SC,
                    )

                t_c = tmp.tile([16, H2], F32, tag="tc2")
                hT = tmp.tile([16, H2], BF16, tag="hT2")
                for hf in range(2):
                    s = slice(hf * 512, (hf + 1) * 512)
                    if t == 0:
                        nc.vector.tensor_mul(c_sb[:, s], t_i[:, s], t_g[:, s])
                    else:
                        nc.vector.tensor_mul(t_f[:, s], t_f[:, s], c_sb[:, s])
                        nc.vector.tensor_mul(t_i[:, s], t_i[:, s], t_g[:, s])
                        nc.vector.tensor_add(c_sb[:, s], t_f[:, s], t_i[:, s])
                    nc.scalar.activation(t_c[:, s], c_sb[:, s], TANH)
                    nc.vector.tensor_mul(hT[:, s], t_o[:, s], t_c[:, s])

                ptr = ptrp.tile([128, 8, BS], BF16, tag="ptr")
                for hc in range(8):
                    nc.tensor.transpose(
                        ptr[:, hc], hT[:, hc * 128 : (hc + 1) * 128], id16_sb[:]
                    )
                nc.vector.tensor_copy(hs_sb[:, :, t, :], ptr[:])
                h8_prev = h8p.tile([128, 8, BS], FP8, tag="h8")
                nc.scalar.activation(h8_prev[:], ptr[:], COPY, scale=HSC)

                # ---- phase 3 (interleaved): chunked AllGather of hs ----
                if t % TPC == TPC - 1:
                    k = t // TPC
                    ts = slice(k * TPC, (k + 1) * TPC)
                    nc.gpsimd.dma_start(out=ag_in[k][:], in_=hs_sb[:, :, ts, :])
                    nc.gpsimd.collective_compute(
                        "AllGather",
                        mybir.AluOpType.bypass,
                        replica_groups=[list(range(NC_N))],
                        ins=[ag_in[k].opt()],
                        outs=[ag_out[k].opt()],
                    )
                    hs_k = hap.tile([128, 8, NC_N, TPC, BS], BF16, tag="ha")
                    hs_all_t.append(hs_k)
                    nc.gpsimd.dma_start(
                        out=hs_k[:],
                        in_=ag_out[k][:].rearrange(
                            "(r p) hc t b -> p hc r t b", p=128
                        ),
                    )

            # ---- phase 4: vocab-sharded projection, pipelined per t-phase ----
            # t-phase ph only needs AllGather chunk ph, so early phases overlap
            # the LSTM; W_out tiles are re-streamed per (phase, vt).
            for ph in range(KC):
                for vt in range(NVT_C):
                    po = pop.tile([128, NC_N, TPC, BS], F32, tag="po")
                    for hc in range(8):
                        nc.tensor.matmul(
                            po[:],
                            wop_all[:, vt, hc],
                            hs_all_t[ph][:, hc],
                            start=(hc == 0),
                            stop=(hc == 7),
                        )
                    st = stgp.tile([128, NC_N, TPC, BS], BF16, tag="st")
                    nc.scalar.activation(
                        st[:], po[:], IDENT, bias=boutT_sb[:, vt : vt + 1]
                    )
                    eng = nc.scalar if ph < 3 else nc.sync
                    eng.dma_start(out_d[ph][vt], st[:])

    nc.compile()
    return nc


def prep_host(features, captions, pad_idx, emb, W_ih, W_hh, b_ih, b_hh, W_out, b_out):
    """Host-side layout prep. Returns (shared dict, per-core list of dicts)."""
    features = np.asarray(features, dtype=np.float32)
    captions = np.asarray(captions).astype(np.int64)
    pad_idx = int(np.asarray(pad_idx))
    emb = np.asarray(emb, dtype=np.float32)
    W_ih = np.asarray(W_ih, dtype=np.float32)
    W_hh = np.asarray(W_hh, dtype=np.float32)
    bsum = np.asarray(b_ih, dtype=np.float32) + np.asarray(b_hh, dtype=np.float32)
    W_out = np.asarray(W_out, dtype=np.float32)
    b_out = np.asarray(b_out, dtype=np.float32)

    # seqtok[t, b]: pad for t=0, captions[b, t-1] for t>=1
    seqtok = np.empty((T, B), np.int64)
    seqtok[0, :] = pad_idx
    seqtok[1:, :] = captions[:, : T - 1].T
    xtok = emb[seqtok]  # [T, B, E]

    # wihT[p, ec, g] = W_ih[g, ec*128+p] * SC
    wihT = np.ascontiguousarray(
        (W_ih * SC).T.reshape(8, 128, G).transpose(1, 0, 2).astype(bf16)
    )
    # whh8[p, j, kt, g] = W_hh[g, (2j+kt)*128+p] * WSC
    whh8 = np.ascontiguousarray(
        (W_hh * WSC).T.reshape(4, 2, 128, G).transpose(2, 0, 1, 3).astype(fp8)
    )
    brow = np.ascontiguousarray((bsum * SC)[None, :].astype(bf16))
    onesrow = np.ones((1, 128), bf16)
    ident16 = np.eye(16, dtype=bf16)

    Wout_pad = np.zeros((VP, H2), np.float32)
    Wout_pad[:V] = W_out
    bout_pad = np.zeros((VP,), np.float32)
    bout_pad[:V] = b_out

    shared = {"wihT": wihT, "whh8": whh8, "brow": brow, "onesrow": onesrow,
              "ident16": ident16}

    per_core = []
    for c in range(NC_N):
        bsl = slice(c * BS, (c + 1) * BS)
        # xstat[p, tblk, ec, ti*16+bl] = xtok[tblk*8+ti, c*16+bl, ec*128+p]
        xs = xtok[:, bsl, :]  # [24, 16, 512]
        xs = xs.reshape(TB, 8, BS, 4, 128)  # [tblk, ti, bl, ec, p]
        xstat = np.ascontiguousarray(
            xs.transpose(4, 0, 3, 1, 2).reshape(128, TB, 4, 8 * BS).astype(bf16)
        )
        # featst[p, ec, ti*16+bl] = features[c*16+bl, ec*128+p]
        f = features[bsl].reshape(BS, 4, 128)  # [bl, ec, p]
        featst = np.ascontiguousarray(
            np.broadcast_to(
                f.transpose(2, 1, 0)[:, :, None, :], (128, 4, 8, BS)
            ).reshape(128, 4, 128).astype(bf16)
        )
        # wop[vt, p(hu), hc, j] = Wout_pad[c*3840 + vt*128 + j, hc*128 + p]
        w = Wout_pad[c * VSH : (c + 1) * VSH].reshape(NVT_C, 128, 8, 128)
        wop = np.ascontiguousarray(w.transpose(0, 3, 2, 1).astype(bf16))
        # boutT[p, vt] = bout_pad[c*3840 + vt*128 + p]
        bT = np.ascontiguousarray(
            bout_pad[c * VSH : (c + 1) * VSH].reshape(NVT_C, 128).T
        )
        per_core.append({"xstat": xstat, "featst": featst, "wop": wop, "boutT": bT})
    return shared, per_core


_NC_CACHE = None


def kernel(**inputs) -> np.ndarray:
    global _NC_CACHE
    if _NC_CACHE is None:
        _NC_CACHE = build_nc()
    nc = _NC_CACHE

    shared, per_core = prep_host(**inputs)
    in_maps = [dict(shared, **pc) for pc in per_core]
    res = run_bass_kernel_spmd(nc, in_maps, core_ids=list(range(NC_N)))

    out = np.empty((B, VP, T), np.float32)
    for c in range(NC_N):
        for ph in range(KC):
            o = np.asarray(res.results[c][f"out{ph}"])  # [30, 128, 8r, TPCt, 16b]
            a = o.astype(np.float32).transpose(2, 4, 0, 1, 3)  # [r, bl, vt, j, tp]
            out[:, c * VSH : (c + 1) * VSH, ph * TPC : (ph + 1) * TPC] = a.reshape(
                B, VSH, TPC
            )
    return out[:, :V, :]
